# revision 14
# baseline (speedup 1.0000x reference)
"""Multi-head attention (B=4, T=2048, C=1024, H=16, D=64) on 8 TRN2 NeuronCores.

Sharding: data-parallel over the 4 batches x tensor-parallel over 2 head
groups (8 heads each).  Core c handles batch (c % 4), head group (c // 4).

Per-core kernel:
  QKV projections in fp16 (fp32 PSUM accumulation).  Q and K are written to
  SBUF as fp8(e4m3) in a DoubleRow pair layout [32*h' + dlo, (dhalf, t)]
  (weight columns pre-permuted on the host), so each score matmul is one
  fp8 DoubleRow instruction: K=32 partitions x 2 k-halves contract the full
  d=64 at 0.5 cycles/row - half the PE cost of fp16 scores.
  P = exp(S/8) on the ACT engine (fp16 out), which is the ~267us critical
  path (33.5M exps/core at 1.2G elem/s/partition).
  AV keeps fp16 with the ones-column trick (row 64 = softmax sums), in 4-kt
  PSUM groups added into an SBUF accumulator (DVE) so PSUM rings stay short.
  Out-projection in fp16 -> oT partials fp32 -> HBM.

  Emission is slot-paced: each of the 256 (pass, kt) score+exp steps pumps
  ~2048 PE-cycles of filler work (projection chains, AV groups, outproj)
  from an ordered queue with earliest-slot pacing, keeping the ACT engine
  saturated while PE fills the gaps.

Host: out[b] = (oT(b, g0) + oT(b, g1)).T + bo + Wo @ bv
(the V-bias contributes exactly Wo @ bv per row because softmax rows sum to 1).
"""

import sys

if "/opt/trn_rl_repo" not in sys.path:
    sys.path.insert(0, "/opt/trn_rl_repo")

import numpy as np
import ml_dtypes

from concourse.bacc import Bacc
import concourse.mybir as mybir
import concourse.tile as tile
from concourse.bass_utils import run_bass_kernel_spmd

F32 = mybir.dt.float32
F16 = mybir.dt.float16  # fp16: same PE speed as bf16, more mantissa
F8 = mybir.dt.float8e4  # e4m3, max 240
EXPF = mybir.ActivationFunctionType.Exp
DR = mybir.MatmulPerfMode.DoubleRow

B, T, C = 4, 2048, 1024
H, D = 16, 64
HPC = 8          # heads per core
CS = HPC * D     # c_out slice per core = 512
NKT = T // 128   # 16 k-tiles over t_k
P_BUFS = 42

# pass order: g0 head-pairs first so group-1 projections can lag; (3,3) last
PASSES = [(0, 0), (1, 0), (0, 1), (1, 1), (2, 0), (3, 0), (2, 1), (3, 1),
          (0, 2), (1, 2), (0, 3), (1, 3), (2, 2), (3, 2), (2, 3), (3, 3)]


def build_nc():
    nc = Bacc(trn_type="TRN2")
    xT_d = nc.dram_tensor("xT", [C, T], F16, kind="ExternalInput")
    wq_d = nc.dram_tensor("wqT", [C, CS], F16, kind="ExternalInput")  # cols permuted
    wk_d = nc.dram_tensor("wkT", [C, CS], F16, kind="ExternalInput")  # cols permuted
    wv_d = nc.dram_tensor("wvT", [C, CS], F16, kind="ExternalInput")
    wo_d = nc.dram_tensor("woT", [CS, C], F16, kind="ExternalInput")
    bq_d = nc.dram_tensor("bq", [CS, 1], F32, kind="ExternalInput")  # permuted
    bk_d = nc.dram_tensor("bk", [CS, 1], F32, kind="ExternalInput")  # permuted
    oT_d = nc.dram_tensor("oT", [C, T], F32, kind="ExternalOutput")

    with tile.TileContext(nc) as tc:
        with (
            tc.tile_pool(name="consts", bufs=1) as consts,
            tc.tile_pool(name="qkv", bufs=1) as qkv,
            tc.tile_pool(name="ptiles", bufs=P_BUFS) as ppool,
            tc.tile_pool(name="small", bufs=2) as small,
            tc.tile_pool(name="ostage", bufs=2) as ostage,
            tc.tile_pool(name="ps", bufs=2, space="PSUM") as ps,
        ):
            # ---- persistent tiles.  One big DMA per tensor (the 625ns HWDGE
            # fixed cost per dma_start is serial), priority-ordered: biases,
            # wk, x halves, wq, then wv/wo; k/q t0-chains unblock at ~13us.
            bq_sb = consts.tile([128, 4], F32)
            nc.sync.dma_start(out=bq_sb, in_=bq_d[:, :].rearrange("(c p) n -> p (c n)", p=128))
            bk_sb = consts.tile([128, 4], F32)
            nc.sync.dma_start(out=bk_sb, in_=bk_d[:, :].rearrange("(c p) n -> p (c n)", p=128))
            wk_sb = consts.tile([128, 8, CS], F16)
            nc.sync.dma_start(out=wk_sb, in_=wk_d[:, :].rearrange("(c p) n -> p c n", p=128))
            x_sb = consts.tile([128, 8, T], F16)
            nc.sync.dma_start(out=x_sb[:, 0:4, 0:512],
                              in_=xT_d[0:512, 0:512].rearrange("(c p) n -> p c n", p=128))
            nc.sync.dma_start(out=x_sb[:, 4:8, 0:512],
                              in_=xT_d[512:1024, 0:512].rearrange("(c p) n -> p c n", p=128))
            wq_sb = consts.tile([128, 8, CS], F16)
            nc.sync.dma_start(out=wq_sb, in_=wq_d[:, :].rearrange("(c p) n -> p c n", p=128))
            nc.sync.dma_start(out=x_sb[:, :, 512:2048],
                              in_=xT_d[:, 512:2048].rearrange("(c p) n -> p c n", p=128))
            wv_sb = consts.tile([128, 8, CS], F16)
            nc.sync.dma_start(out=wv_sb, in_=wv_d[:, :].rearrange("(c p) n -> p c n", p=128))
            wo_sb = consts.tile([128, 4, C], F16)
            nc.sync.dma_start(out=wo_sb, in_=wo_d[:, :].rearrange("(c p) n -> p c n", p=128))
            xts = [x_sb[:, ci, :] for ci in range(8)]
            wk_cis = [wk_sb[:, ci, :] for ci in range(8)]
            wq_cis = [wq_sb[:, ci, :] for ci in range(8)]

            # q8/k8: per 4-head group g, [32*h' + dlo, dhalf, t] fp8
            q8s = [qkv.tile([128, 2, T], F8, tag=f"q8{g}", name=f"q8{g}") for g in range(2)]
            k8s = [qkv.tile([128, 2, T], F8, tag=f"k8{g}", name=f"k8{g}") for g in range(2)]
            # V with a ones column appended: vh_tts[tt] = [tk part, head, 64+1]
            vh_tts = []
            for tt in range(NKT):
                vht = qkv.tile([128, HPC, D + 1], F16, tag=f"vh{tt}", name=f"vh{tt}")
                nc.vector.memset(vht[:, :, D:D + 1], 1.0)
                vh_tts.append(vht)
            # attention output per q-chunk, ring of 2 (freed by outproj)
            a_qcs = {}
            # SBUF AV accumulators (short PSUM group -> SBUF add), ring of 4
            acc_sbs = {}

            ptiles = {}   # pass index -> list of 16 P tiles
            state = {"slot": 0, "budget": 0.0, "qi": 0}
            queue = []    # (min_slot, cost, fn) in order

            # ---- unit bodies ----
            def qk_chain(w_cis, b_sb, dsts, g, dh, t):
                mt = 2 * g + dh
                def fn():
                    pmm = ps.tile([128, 512], F32, tag="acc", bufs=2, name="pmm")
                    for ci in range(8):
                        nc.tensor.matmul(
                            pmm,
                            w_cis[ci][:, mt * 128:(mt + 1) * 128],
                            xts[ci][:, t * 512:(t + 1) * 512],
                            start=(ci == 0), stop=(ci == 7),
                        )
                    nc.vector.tensor_scalar_add(
                        dsts[g][:, dh, t * 512:(t + 1) * 512], pmm, b_sb[:, mt:mt + 1]
                    )
                return fn

            def v_unit(tt):
                def fn():
                    pmm = ps.tile([128, 512], F32, tag="acc", bufs=2, name="vproj")
                    for ci in range(8):
                        nc.tensor.matmul(
                            pmm,
                            xts[ci][:, tt * 128:(tt + 1) * 128],
                            wv_sb[:, ci, :],
                            start=(ci == 0), stop=(ci == 7),
                        )
                    for h in range(HPC):
                        nc.vector.tensor_copy(
                            vh_tts[tt][:, h, 0:D], pmm[:, h * D:(h + 1) * D]
                        )
                return fn

            def av_unit(pi, hb, g):
                hp, qc = PASSES[pi]
                h = 2 * hp + hb
                last = pi == len(PASSES) - 1
                def fn():
                    key = (pi, hb)
                    if last:
                        # final pass: one full-pass PSUM chain; normalize reads
                        # PSUM directly, skipping the serial SBUF-add chain
                        # that would otherwise sit between the last exp and
                        # the last out-projection.
                        if g == 0:
                            acc_sbs[key] = ps.tile([65, 512], F32, tag="av",
                                                   bufs=2, name=f"avL{hb}")
                        av = acc_sbs[key]
                        st, sp = g == 0, g == 3
                    else:
                        av = ps.tile([65, 512], F32, tag="av", bufs=2,
                                     name=f"av{pi}_{hb}_{g}")
                        st, sp = True, True
                    for kt in range(4 * g, 4 * g + 4):
                        nc.tensor.matmul(
                            av,
                            vh_tts[kt][:, h, :],
                            ptiles[pi][kt][:, hb * 512:(hb + 1) * 512],
                            start=(st and kt == 4 * g), stop=(sp and kt == 4 * g + 3),
                        )
                    if not last:
                        if g == 0:
                            acc_sbs[key] = small.tile([65, 512], F16, tag="avacc",
                                                      bufs=4, name=f"acc{pi}_{hb}")
                            nc.vector.tensor_copy(acc_sbs[key], av)
                        else:
                            nc.vector.tensor_add(acc_sbs[key], acc_sbs[key], av)
                    if g == 3:
                        normalize(pi, hb, acc_sbs[key])
                return fn

            def normalize(pi, hb, acc):
                hp, qc = PASSES[pi]
                if qc not in a_qcs:
                    a_qcs[qc] = qkv.tile([128, 4, 512], F16, tag="aq",
                                         bufs=2, name=f"a{qc}")
                ssum = small.tile([1, 512], F32, tag="ssum", bufs=1, name="ssum")
                nc.vector.tensor_copy(ssum, acc[64:65, :])
                rec = small.tile([1, 512], F32, tag="rec", bufs=1, name="rec")
                nc.vector.reciprocal_approx_fast(out=rec, in_=ssum)
                rbs = small.tile([64, 512], F32, tag="rbs", bufs=1, name="rbs")
                nc.gpsimd.partition_broadcast(rbs, rec)
                nc.vector.tensor_mul(
                    a_qcs[qc][hb * 64:hb * 64 + 64, hp, :], acc[0:64, :], rbs
                )

            def o_unit(qc, mt):
                def fn():
                    po = ps.tile([128, 512], F32, tag="acc", bufs=2, name="po")
                    for ci in range(4):
                        nc.tensor.matmul(
                            po,
                            wo_sb[:, ci, mt * 128:(mt + 1) * 128],
                            a_qcs[qc][:, ci, :],
                            start=(ci == 0), stop=(ci == 3),
                        )
                    ot = ostage.tile([128, 512], F32, tag="ot", bufs=2, name="ot")
                    nc.vector.tensor_copy(ot, po)
                    nc.sync.dma_start(
                        out=oT_d[mt * 128:(mt + 1) * 128, qc * 512:(qc + 1) * 512],
                        in_=ot,
                    )
                return fn

            # ---- scores + exp step (one kt of one pass) ----
            def scores_exp(pi, hp, qc, kt):
                g, q0 = hp // 2, qc * 512
                sc = ps.tile([128, 1024], F32, tag="sc", bufs=2, name="sc")
                for hb in range(2):
                    hh = 2 * hp + hb - 4 * g
                    nc.tensor.matmul(
                        sc[:, hb * 512:(hb + 1) * 512],
                        k8s[g][32 * hh:32 * hh + 32, :, kt * 128:(kt + 1) * 128],
                        q8s[g][32 * hh:32 * hh + 32, :, q0:q0 + 512],
                        start=True, stop=True, perf_mode=DR,
                        tile_position=(32 * hh, 0),
                    )
                pt = ppool.tile([128, 1024], F16, tag="P", bufs=P_BUFS, name="pt")
                nc.scalar.activation(pt, sc, EXPF, scale=0.125)
                ptiles[pi].append(pt)

            # ---- build the filler queue (order is dependency-safe; AV units
            # additionally gated by earliest-slot = when their P tiles exist).
            def enq(fn, cost=1, min_slot=0):
                queue.append((min_slot, cost, fn))

            def enq_av(pi):
                for g in range(4):
                    for hb in range(2):
                        enq(av_unit(pi, hb, g), 1, 16 * pi + 4 * g + 4)

            def enq_av_grp(pi, g):
                for hb in range(2):
                    enq(av_unit(pi, hb, g), 1, 16 * pi + 4 * g + 4)

            for t in range(1, 4):
                for dh in range(2):
                    enq(qk_chain(wk_cis, bk_sb, k8s, 0, dh, t), 2)   # k(g0,t1-3)
            for dh in range(2):
                enq(qk_chain(wq_cis, bq_sb, q8s, 0, dh, 1), 2)       # q(g0,qc1)
            for dh in range(2):
                for t in range(4):
                    enq(qk_chain(wk_cis, bk_sb, k8s, 1, dh, t), 2)   # k(g1)
            for dh in range(2):
                enq(qk_chain(wq_cis, bq_sb, q8s, 1, dh, 0), 2)       # q(g1,qc0)
            for tt in range(4):
                enq(v_unit(tt), 2)
            enq_av_grp(0, 0)
            for tt in range(4, 8):
                enq(v_unit(tt), 2)
            enq_av_grp(0, 1)
            enq_av_grp(1, 0)
            for tt in range(8, 12):
                enq(v_unit(tt), 2)
            enq_av_grp(0, 2)
            enq_av_grp(1, 1)
            for tt in range(12, 16):
                enq(v_unit(tt), 2)
            enq_av_grp(0, 3)
            enq_av_grp(1, 2)
            enq_av_grp(1, 3)
            enq_av(2)
            for dh in range(2):
                enq(qk_chain(wq_cis, bq_sb, q8s, 1, dh, 1), 2)       # q(g1,qc1)
            enq_av(3)
            for mt in range(8):
                enq(o_unit(0, mt))
            for dh in range(2):
                enq(qk_chain(wq_cis, bq_sb, q8s, 0, dh, 2), 2)       # q(g0,qc2)
            enq_av(4)
            for dh in range(2):
                enq(qk_chain(wq_cis, bq_sb, q8s, 1, dh, 2), 2)       # q(g1,qc2)
            enq_av(5)
            for dh in range(2):
                enq(qk_chain(wq_cis, bq_sb, q8s, 0, dh, 3), 2)       # q(g0,qc3)
            enq_av(6)
            for dh in range(2):
                enq(qk_chain(wq_cis, bq_sb, q8s, 1, dh, 3), 2)       # q(g1,qc3)
            enq_av(7)
            for mt in range(8):
                enq(o_unit(1, mt))
            for pi in range(8, 12):
                enq_av(pi)
            for pi in range(12, 14):
                enq_av(pi)
            for mt in range(6):
                enq(o_unit(2, mt))
            enq_av(14)
            for g in range(3):
                enq_av_grp(15, g)
            for mt in range(6, 8):
                enq(o_unit(2, mt))
            enq_av_grp(15, 3)
            for mt in range(8):
                enq(o_unit(3, mt))

            def pump():
                state["budget"] = min(state["budget"] + 1, 3.0)
                while state["qi"] < len(queue):
                    ms, cost, fn = queue[state["qi"]]
                    if ms > state["slot"] or cost > state["budget"]:
                        break
                    fn()
                    state["qi"] += 1
                    state["budget"] -= cost

            # ---- head: k(g0) + q(g0, qc0), then the slot-paced main loop
            for dh in range(2):
                qk_chain(wk_cis, bk_sb, k8s, 0, dh, 0)()
            for dh in range(2):
                qk_chain(wq_cis, bq_sb, q8s, 0, dh, 0)()

            for pi, (hp, qc) in enumerate(PASSES):
                ptiles[pi] = []
                for kt in range(NKT):
                    scores_exp(pi, hp, qc, kt)
                    state["slot"] += 1
                    pump()
            # tail: drain remaining units (o(3) and any stragglers)
            while state["qi"] < len(queue):
                queue[state["qi"]][2]()
                state["qi"] += 1
    nc.finalize()
    return nc


_NC = None


def _get_nc():
    global _NC
    if _NC is None:
        _NC = build_nc()
    return _NC


def _qk_col_perm():
    """perm[new] = old column index within a core's 512-col slice: block
    mt' = 2g+dh holds partition 32h'+dlo = head (4g+h'), d = 32dh+dlo."""
    perm = np.empty(CS, dtype=np.int64)
    i = 0
    for g in range(2):
        for dh in range(2):
            for hh in range(4):
                for dlo in range(32):
                    perm[i] = (4 * g + hh) * 64 + 32 * dh + dlo
                    i += 1
    return perm


_PERM = _qk_col_perm()


def _shard_inputs(x, Wq, bq, Wk, bk, Wv, bv, Wo, bo):
    f16 = np.float16
    x = np.asarray(x, np.float32)
    in_maps = []
    wqT = np.ascontiguousarray(np.asarray(Wq, np.float32).T).astype(f16)  # [c_in, c_out]
    wkT = np.ascontiguousarray(np.asarray(Wk, np.float32).T).astype(f16)
    wvT = np.ascontiguousarray(np.asarray(Wv, np.float32).T).astype(f16)
    woT = np.ascontiguousarray(np.asarray(Wo, np.float32).T).astype(f16)
    xT = [np.ascontiguousarray(x[b].T).astype(f16) for b in range(B)]
    bq = np.asarray(bq, np.float32)
    bk = np.asarray(bk, np.float32)
    for c in range(8):
        b, g = c % B, c // B
        sl = slice(g * CS, (g + 1) * CS)
        in_maps.append({
            "xT": xT[b],
            "wqT": np.ascontiguousarray(wqT[:, sl][:, _PERM]),
            "wkT": np.ascontiguousarray(wkT[:, sl][:, _PERM]),
            "wvT": np.ascontiguousarray(wvT[:, sl]),
            "woT": np.ascontiguousarray(woT[sl, :]),
            "bq": np.ascontiguousarray(bq[sl][_PERM]).reshape(CS, 1),
            "bk": np.ascontiguousarray(bk[sl][_PERM]).reshape(CS, 1),
        })
    return in_maps


def run_sharded(inputs, **kwargs):
    nc = _get_nc()
    in_maps = _shard_inputs(**inputs)
    return run_bass_kernel_spmd(nc, in_maps, core_ids=list(range(8)), **kwargs)


def assemble(results, Wv_bias, Wo, bo):
    bo_eff = (np.asarray(bo, np.float32)
              + np.asarray(Wo, np.float32) @ np.asarray(Wv_bias, np.float32))
    out = np.empty((B, T, C), np.float32)
    for b in range(B):
        acc = results[b]["oT"].astype(np.float32) + results[b + B]["oT"].astype(np.float32)
        out[b] = acc.T + bo_eff[None, :]
    return out


def kernel(**inputs):
    res = run_sharded(inputs)
    return assemble(res.results, inputs["bv"], inputs["Wo"], inputs["bo"])
